# revision 22
# baseline (speedup 1.0000x reference)
"""Trainium2 Bass kernel for nn_Encoder (R-GCN style message passing).

Math (faithful to the reference, including its s-major/f-major index mismatch):
    supports_ = concat_s(A[s] @ features)            # [N, S*F], cols k=s*F+f
    Vmat      = (W_comp @ W.transpose(1,0,2)).reshape(S*F, E)   # rows k=f*S+s
    out       = supports_ @ Vmat

Rewritten as one big contraction:
    Q_s[f, e]  = Vmat[s*F + f, e]        (contiguous 32-row block of Vmat)
    H_s        = features @ Q_s          # [N, E]  (tiny: 8.4 MFLOP)
    out        = sum_s A[s] @ H_s        # 17.2 GFLOP, all on device
    i.e. with Acat[(s,m), n] = A[s, n, m]; Hcat[(s,m), e] = H_s[m, e]:
    out.T      = Hcat.T @ Acat

Sharding: node dim N split across 8 cores (1024 rows each). Each core
streams its A-shard through the PE as the moving operand.

Layout/perf choices (v3):
  * A is stored in HBM as float8_e3m4 (1 byte/elem, host-quantized with a
    global scale folded into Hcat), halving HBM traffic vs fp16. Output
    median rel err ~1.3e-2, from quantizing A; well under the 2e-2 gate.
  * The main matmul is 4x column-tiled: four K-chunks stream concurrently
    through the four 32-column groups of the PE array (stationary H chunks
    are [128, 32]), each accumulating into its own 32-partition slice of
    the PSUM banks. A final selector-matmul (4 stacked 32x32 identities)
    reduces the four partition groups to the [32, 1024] out.T. Without the
    tiling the PE (1 moving col/cycle) would bottleneck at ~109us.
  * Hcat (the tiny 8.4-MFLOP features @ Q product, 0.05% of total FLOPs)
    is precomputed on host alongside the layout transposes and uploaded as
    a 2 MB fp16 constant. Computing it on device costs ~0.5us of PE per
    A-block, which at the HAM-throttled half clock (K=4/8) pushes the PE
    past the per-block DMA time and stalls the stream; with it removed the
    PE keeps a >20% margin even fully cold and the kernel stays DMA-bound
    end-to-end (~420 GB/s measured).
  * Deep A-buffer ring (16 x 1MB) absorbs HAM K=4/8 transients so the DMA
    stream never backpressures; A-block DMAs are issued before constants
    so first bytes land as soon as the framework preamble ends.
"""

import os
import numpy as np
import ml_dtypes

import concourse.bass as bass
import concourse.mybir as mybir
from concourse import bacc, bass_utils
from concourse.tile import TileContext

S, N, F, E = 4, 8192, 32, 32
P = 128
N_CORES = 8
NS = N // N_CORES          # 1024 node rows per core
KTOT = S * N               # 32768 contraction rows
NCHUNK = KTOT // P         # 256 K-chunks of 128
JPB = int(os.environ.get("KJPB", "8"))   # K-chunks per DMA block
NBLK = NCHUNK // JPB       # DMA blocks
MB = N // (P * JPB)        # DMA blocks per relation
TAILB = int(os.environ.get("KTAILB", "4"))  # ring-independent final blocks

# Matmul dtype for the big streaming matmul ('e3m4' | 'fp16').
MAIN_DT = os.environ.get("KDT", "e3m4")

_DT_MAP = {
    "e3m4": (mybir.dt.float8e3, ml_dtypes.float8_e3m4),
    "fp16": (mybir.dt.float16, np.float16),
}
E3M4_MAX = 15.0   # target absmax after scaling (format max 15.5)


def _build(dt_key):
    """Build + finalize the per-core Bass program (same program on all cores)."""
    dt_main, _ = _DT_MAP[dt_key]
    f32 = mybir.dt.float32
    f32r = mybir.dt.float32r
    fp16 = mybir.dt.float16
    abufs = int(os.environ.get("KABUFS", "12" if JPB == 8 else "24"))

    nc = bacc.Bacc("TRN2")
    atc = nc.dram_tensor("atc", [KTOT, NS], dt_main, kind="ExternalInput")
    # hcatT[p, c*E+e] = H[k, e] for contraction row k = (c//JPB)*(P*JPB)
    #                 + p*JPB + (c%JPB), matching atc's row permutation
    hcatT = nc.dram_tensor("hcatT", [P, NCHUNK * E], fp16, kind="ExternalInput")
    # 4 stacked 32x32 identities: reduces the 4 column-group partials
    sel = nc.dram_tensor("sel", [P, E], f32r, kind="ExternalInput")
    outT = nc.dram_tensor("outT", [E, NS], f32, kind="ExternalOutput")

    # Contraction rows permuted so partition p's block data is one contiguous
    # run: row k = b*(P*JPB) + p*JPB + j  (8 KB per partition per DMA).
    atc_r = atc.rearrange("(b p j) n -> b p (j n)", p=P, j=JPB)

    with TileContext(nc) as tc:
        with (
            tc.tile_pool(name="consts", bufs=1) as consts,
            tc.tile_pool(name="abuf", bufs=abufs) as apool,
            tc.tile_pool(name="tailb", bufs=1) as tailpool,
            tc.tile_pool(name="ops", bufs=1, space="PSUM") as opsum,
            tc.tile_pool(name="redps", bufs=1, space="PSUM") as redps,
            tc.tile_pool(name="osb", bufs=1) as osb,
        ):
            # A-block loads rotate across THREE DMA-issue rings (SP/sync,
            # ACT/scalar HWDGE queues + the gpsimd dynamic queue — gpsimd is
            # otherwise idle here). More rings = faster issue ramp at
            # startup and more DMA instructions pending when the stream
            # drains, which keeps the trailing blocks from trickling out of
            # a lone queue.
            rings = (nc.sync, nc.scalar, nc.gpsimd)

            def a_dma(b, ab):
                rings[b % 3].dma_start(ab, atc_r[b])

            # ---- first A-block loads, then constants: the A stream is the
            # critical path so its first blocks get the rings first; the
            # consts land a few us later, well before the PE needs them.
            pre = {}
            for b in range(min(6, NBLK)):
                ab = apool.tile([P, JPB * NS], dt_main)
                a_dma(b, ab)
                pre[b] = ab
            # The last TAILB blocks get dedicated buffers, issued up front
            # with no ring-slot dependency, and are CONSUMED early (the
            # accumulation is order-free). The endgame then owes the stream
            # only one in-flight block instead of a ring-gated backlog —
            # the ring's trailing issues are serialized behind PE-progress
            # semaphores and their delivery decays exactly when the PE is
            # least able to absorb it (HAM-cold).
            for b in range(NBLK - TAILB, NBLK):
                tb = tailpool.tile([P, JPB * NS], dt_main, tag=f"tail{b}")
                a_dma(b, tb)
                pre[b] = tb

            # constants ride the other ring (ACT/scalar) so they never
            # interleave with the A stream's queue
            hcat = consts.tile([P, NCHUNK * E], fp16)
            half = NCHUNK * E // 2
            nc.scalar.dma_start(hcat[:, 0:half], hcatT[:, 0:half])
            nc.scalar.dma_start(hcat[:, half:], hcatT[:, half:])
            selt = consts.tile([P, E], f32r)
            nc.scalar.dma_start(selt, sel[:, :])

            # ---- main streaming matmul, 4x column-tiled ----
            # chunk c feeds column group (c % 4); its [128, 32] stationary H
            # sits in array columns 32j..32j+31 and accumulates into PSUM
            # partitions 32j..32j+31. Four chunks stream concurrently.
            psA = opsum.tile([P, 512], f32)
            psB = opsum.tile([P, 512], f32)

            # consumption order: preloaded blocks, then the early-resident
            # tail blocks, then the ring-streamed middle; the last-consumed
            # block is the last one the stream delivers
            order = list(range(min(6, NBLK)))
            order += list(range(NBLK - TAILB, NBLK))
            order += list(range(min(6, NBLK), NBLK - TAILB))
            nmm = {}
            for bi, b in enumerate(order):
                if b in pre:
                    ab = pre.pop(b)
                else:
                    ab = apool.tile([P, JPB * NS], dt_main)
                    a_dma(b, ab)
                if bi < len(order) - 1:
                    waves = [(jj, ps) for jj in range(0, JPB, 4)
                             for ps in (psA, psB)]
                else:
                    # last block: close psA's accumulation first so its
                    # copy+reduce+store overlaps psB's final waves
                    waves = [(jj, psA) for jj in range(0, JPB, 4)]
                    waves += [(jj, psB) for jj in range(0, JPB, 4)]
                for jj, ps in waves:
                    off = 0 if ps is psA else 512
                    for j2 in range(4):
                        j = jj + j2
                        c = b * JPB + j
                        k = (id(ps), j2)
                        cnt = nmm.get(k, 0)
                        nmm[k] = cnt + 1
                        nc.tensor.matmul(
                            ps[32 * j2 : 32 * (j2 + 1), :],
                            hcat[:, c * E : (c + 1) * E],
                            ab[:, j * NS + off : j * NS + off + 512],
                            start=(cnt == 0),
                            stop=(cnt == NCHUNK // 4 - 1),
                            skip_group_check=True,
                            tile_position=(0, 32 * j2),
                        )

            # ---- tail: PSUM->SBUF, 4-group partition reduction, store ----
            # split halves across engines + both HWDGE rings so the psA
            # half's copy+reduce+store overlaps the psB half's
            sbA = osb.tile([P, 512], f32r, tag="sbA")
            sbB = osb.tile([P, 512], f32r, tag="sbB")
            nc.scalar.copy(sbA, psA)
            nc.vector.tensor_copy(sbB, psB)
            redA = redps.tile([E, 512], f32)
            redB = redps.tile([E, 512], f32)
            nc.tensor.matmul(redA, selt, sbA, start=True, stop=True)
            nc.tensor.matmul(redB, selt, sbB, start=True, stop=True)
            ot0 = osb.tile([E, 512], f32, tag="ot0")
            ot1 = osb.tile([E, 512], f32, tag="ot1")
            nc.scalar.copy(ot0, redA)
            nc.vector.tensor_copy(ot1, redB)
            nc.sync.dma_start(outT[:, 0:512], ot0)
            nc.scalar.dma_start(outT[:, 512:NS], ot1)

    nc.finalize()
    return nc


_built_cache = {}


def _get_nc(dt_key):
    if dt_key not in _built_cache:
        _built_cache[dt_key] = _build(dt_key)
    return _built_cache[dt_key]


def _shard_inputs(features, A, W, W_comp, dt_key):
    np_main = _DT_MAP[dt_key][1]
    features = np.asarray(features, dtype=np.float32)
    A = np.asarray(A, dtype=np.float32)
    W = np.asarray(W, dtype=np.float32)
    W_comp = np.asarray(W_comp, dtype=np.float32)

    if dt_key == "e3m4":
        absmax = max(float(A.max()), -float(A.min()))
        s_a = E3M4_MAX / max(absmax, 1e-30)
    else:
        s_a = 1.0

    # Hcat = features @ Q per relation (8.4 MFLOP), with the A-quantization
    # scale folded in: (s_a*A) @ feat @ (Q/s_a) is exactly compensated.
    # Vmat rows are f-major (faithful to the reference's index mismatch);
    # Q_s is its contiguous 32-row block s.
    V = np.einsum("sb,fbe->fse", W_comp, W.transpose(1, 0, 2)).reshape(S * F, E)
    Q = V.reshape(S, F, E) * np.float32(1.0 / s_a)
    H = np.einsum("nf,sfe->sne", features, Q).reshape(KTOT, E)   # row k = s*N+m
    # permute rows k = (b, p, j) -> hcatT[p, ((b, j), e)]
    hcatT = np.ascontiguousarray(
        H.reshape(NBLK, P, JPB, E).transpose(1, 0, 2, 3).reshape(P, NCHUNK * E)
    ).astype(np.float16)
    sel = np.ascontiguousarray(np.tile(np.eye(E, dtype=np.float32), (4, 1)))

    # quantize once in natural layout, then transpose/slice in the 1-byte
    # (or 2-byte) domain — much cheaper than per-shard f32 transposes
    if dt_key == "e3m4":
        Aq = (A * np.float32(s_a)).astype(np_main)
    else:
        Aq = A.astype(np_main)
    AqT = np.ascontiguousarray(Aq.transpose(0, 2, 1))             # [S, M, N]
    in_maps = []
    for c in range(N_CORES):
        atc = np.ascontiguousarray(
            AqT[:, :, c * NS : (c + 1) * NS]
        ).reshape(KTOT, NS)
        in_maps.append({"atc": atc, "hcatT": hcatT, "sel": sel})
    return in_maps


def _run(features, A, W, W_comp, dt_key=None, trace=False):
    dt_key = dt_key or MAIN_DT
    nc = _get_nc(dt_key)
    in_maps = _shard_inputs(features, A, W, W_comp, dt_key)
    res = bass_utils.run_bass_kernel_spmd(
        nc, in_maps, core_ids=list(range(N_CORES)), trace=trace
    )
    out = np.concatenate(
        [res.results[c]["outT"].T for c in range(N_CORES)], axis=0
    ).astype(np.float32)
    return out, res


def kernel(features, A, W, W_comp):
    try:
        out, _ = _run(features, A, W, W_comp)
    except Exception:
        # Rare transient device-unrecoverable flakes: reset jax backends and
        # retry once with a freshly built program.
        import jax
        try:
            jax.clear_caches()
            jax.extend.backend.clear_backends()
        except Exception:
            pass
        _built_cache.clear()
        out, _ = _run(features, A, W, W_comp)
    return out


# revision 23
# speedup vs baseline: 1.0835x; 1.0835x over previous
"""Trainium2 Bass kernel for nn_Encoder (R-GCN style message passing).

Math (faithful to the reference, including its s-major/f-major index mismatch):
    supports_ = concat_s(A[s] @ features)            # [N, S*F], cols k=s*F+f
    Vmat      = (W_comp @ W.transpose(1,0,2)).reshape(S*F, E)   # rows k=f*S+s
    out       = supports_ @ Vmat

Rewritten as one big contraction:
    Q_s[f, e]  = Vmat[s*F + f, e]        (contiguous 32-row block of Vmat)
    H_s        = features @ Q_s          # [N, E]  (tiny: 8.4 MFLOP)
    out        = sum_s A[s] @ H_s        # 17.2 GFLOP, all on device
    i.e. with Acat[(s,m), n] = A[s, n, m]; Hcat[(s,m), e] = H_s[m, e]:
    out.T      = Hcat.T @ Acat

Sharding: node dim N split across 8 cores (1024 rows each). Each core
streams its A-shard through the PE as the moving operand.

Layout/perf choices (v3):
  * A is stored in HBM as float8_e3m4 (1 byte/elem, host-quantized with a
    global scale folded into Hcat), halving HBM traffic vs fp16. Output
    median rel err ~1.3e-2, from quantizing A; well under the 2e-2 gate.
  * The main matmul is 4x column-tiled: four K-chunks stream concurrently
    through the four 32-column groups of the PE array (stationary H chunks
    are [128, 32]), each accumulating into its own 32-partition slice of
    the PSUM banks. A final selector-matmul (4 stacked 32x32 identities)
    reduces the four partition groups to the [32, 1024] out.T. Without the
    tiling the PE (1 moving col/cycle) would bottleneck at ~109us.
  * Hcat (the tiny 8.4-MFLOP features @ Q product, 0.05% of total FLOPs)
    is precomputed on host alongside the layout transposes and uploaded as
    a 2 MB fp16 constant. Computing it on device costs ~0.5us of PE per
    A-block, which at the HAM-throttled half clock (K=4/8) pushes the PE
    past the per-block DMA time and stalls the stream; with it removed the
    PE keeps a >20% margin even fully cold and the kernel stays DMA-bound
    end-to-end (~420 GB/s measured).
  * Deep A-buffer ring (16 x 1MB) absorbs HAM K=4/8 transients so the DMA
    stream never backpressures; A-block DMAs are issued before constants
    so first bytes land as soon as the framework preamble ends.
"""

import os
import numpy as np
import ml_dtypes

import concourse.bass as bass
import concourse.mybir as mybir
from concourse import bacc, bass_utils
from concourse.tile import TileContext

S, N, F, E = 4, 8192, 32, 32
P = 128
N_CORES = 8
NS = N // N_CORES          # 1024 node rows per core
KTOT = S * N               # 32768 contraction rows
NCHUNK = KTOT // P         # 256 K-chunks of 128
JPB = int(os.environ.get("KJPB", "8"))   # K-chunks per DMA block
NBLK = NCHUNK // JPB       # DMA blocks
MB = N // (P * JPB)        # DMA blocks per relation
TAILB = int(os.environ.get("KTAILB", "4"))  # ring-independent final blocks

# Matmul dtype for the big streaming matmul ('e3m4' | 'fp16').
MAIN_DT = os.environ.get("KDT", "e3m4")

_DT_MAP = {
    "e3m4": (mybir.dt.float8e3, ml_dtypes.float8_e3m4),
    "fp16": (mybir.dt.float16, np.float16),
}
E3M4_MAX = 15.0   # target absmax after scaling (format max 15.5)


def _build(dt_key):
    """Build + finalize the per-core Bass program (same program on all cores)."""
    dt_main, _ = _DT_MAP[dt_key]
    f32 = mybir.dt.float32
    f32r = mybir.dt.float32r
    fp16 = mybir.dt.float16
    abufs = int(os.environ.get("KABUFS", "12" if JPB == 8 else "24"))

    nc = bacc.Bacc("TRN2")
    atc = nc.dram_tensor("atc", [KTOT, NS], dt_main, kind="ExternalInput")
    # hcatT[p, c*E+e] = H[k, e] for contraction row k = (c//JPB)*(P*JPB)
    #                 + p*JPB + (c%JPB), matching atc's row permutation
    hcatT = nc.dram_tensor("hcatT", [P, NCHUNK * E], fp16, kind="ExternalInput")
    # 4 stacked 32x32 identities: reduces the 4 column-group partials
    sel = nc.dram_tensor("sel", [P, E], f32r, kind="ExternalInput")
    outT = nc.dram_tensor("outT", [E, NS], f32, kind="ExternalOutput")

    # Contraction rows permuted so partition p's block data is one contiguous
    # run: row k = b*(P*JPB) + p*JPB + j  (8 KB per partition per DMA).
    atc_r = atc.rearrange("(b p j) n -> b p (j n)", p=P, j=JPB)

    with TileContext(nc) as tc:
        with (
            tc.tile_pool(name="consts", bufs=1) as consts,
            tc.tile_pool(name="abuf", bufs=abufs) as apool,
            tc.tile_pool(name="tailb", bufs=1) as tailpool,
            tc.tile_pool(name="ops", bufs=1, space="PSUM") as opsum,
            tc.tile_pool(name="redps", bufs=1, space="PSUM") as redps,
            tc.tile_pool(name="osb", bufs=1) as osb,
        ):
            # A-block loads alternate between the two independent HWDGE
            # rings (SP/sync and ACT/scalar). The trailing ring blocks are
            # split into cross-ring halves: the end-of-stream delivery rate
            # scales with how many DMA instructions are still pending, so
            # finer trailing instructions keep both queues fed to the last
            # byte instead of trickling out of a single drained queue.
            rings = (nc.sync, nc.scalar)
            split_from = max(NBLK - TAILB - 6, 0)

            def a_dma(b, ab):
                if split_from <= b < NBLK - TAILB:
                    h = JPB * NS // 2
                    rings[b % 2].dma_start(ab[:, 0:h], atc_r[b][:, 0:h])
                    rings[(b + 1) % 2].dma_start(ab[:, h:], atc_r[b][:, h:])
                else:
                    rings[b % 2].dma_start(ab, atc_r[b])

            # ---- first A-block loads, then constants: the A stream is the
            # critical path so its first blocks get the rings first; the
            # consts land a few us later, well before the PE needs them.
            pre = {}
            for b in range(min(6, NBLK)):
                ab = apool.tile([P, JPB * NS], dt_main)
                a_dma(b, ab)
                pre[b] = ab
            # The last TAILB blocks get dedicated buffers, issued up front
            # with no ring-slot dependency, and are CONSUMED early (the
            # accumulation is order-free). The endgame then owes the stream
            # only one in-flight block instead of a ring-gated backlog —
            # the ring's trailing issues are serialized behind PE-progress
            # semaphores and their delivery decays exactly when the PE is
            # least able to absorb it (HAM-cold).
            for b in range(NBLK - TAILB, NBLK):
                tb = tailpool.tile([P, JPB * NS], dt_main, tag=f"tail{b}")
                a_dma(b, tb)
                pre[b] = tb

            # constants ride the other ring (ACT/scalar) so they never
            # interleave with the A stream's queue
            hcat = consts.tile([P, NCHUNK * E], fp16)
            half = NCHUNK * E // 2
            nc.scalar.dma_start(hcat[:, 0:half], hcatT[:, 0:half])
            nc.scalar.dma_start(hcat[:, half:], hcatT[:, half:])
            selt = consts.tile([P, E], f32r)
            nc.scalar.dma_start(selt, sel[:, :])

            # ---- main streaming matmul, 4x column-tiled ----
            # chunk c feeds column group (c % 4); its [128, 32] stationary H
            # sits in array columns 32j..32j+31 and accumulates into PSUM
            # partitions 32j..32j+31. Four chunks stream concurrently.
            psA = opsum.tile([P, 512], f32)
            psB = opsum.tile([P, 512], f32)

            # consumption order: preloaded blocks, then the early-resident
            # tail blocks, then the ring-streamed middle; the last-consumed
            # block is the last one the stream delivers
            order = list(range(min(6, NBLK)))
            order += list(range(NBLK - TAILB, NBLK))
            order += list(range(min(6, NBLK), NBLK - TAILB))
            nmm = {}
            for bi, b in enumerate(order):
                if b in pre:
                    ab = pre.pop(b)
                else:
                    ab = apool.tile([P, JPB * NS], dt_main)
                    a_dma(b, ab)
                if bi < len(order) - 1:
                    waves = [(jj, ps) for jj in range(0, JPB, 4)
                             for ps in (psA, psB)]
                else:
                    # last block: close psA's accumulation first so its
                    # copy+reduce+store overlaps psB's final waves
                    waves = [(jj, psA) for jj in range(0, JPB, 4)]
                    waves += [(jj, psB) for jj in range(0, JPB, 4)]
                for jj, ps in waves:
                    off = 0 if ps is psA else 512
                    for j2 in range(4):
                        j = jj + j2
                        c = b * JPB + j
                        k = (id(ps), j2)
                        cnt = nmm.get(k, 0)
                        nmm[k] = cnt + 1
                        nc.tensor.matmul(
                            ps[32 * j2 : 32 * (j2 + 1), :],
                            hcat[:, c * E : (c + 1) * E],
                            ab[:, j * NS + off : j * NS + off + 512],
                            start=(cnt == 0),
                            stop=(cnt == NCHUNK // 4 - 1),
                            skip_group_check=True,
                            tile_position=(0, 32 * j2),
                        )

            # ---- tail: PSUM->SBUF, 4-group partition reduction, store ----
            # split halves across engines + both HWDGE rings so the psA
            # half's copy+reduce+store overlaps the psB half's
            sbA = osb.tile([P, 512], f32r, tag="sbA")
            sbB = osb.tile([P, 512], f32r, tag="sbB")
            nc.scalar.copy(sbA, psA)
            nc.vector.tensor_copy(sbB, psB)
            redA = redps.tile([E, 512], f32)
            redB = redps.tile([E, 512], f32)
            nc.tensor.matmul(redA, selt, sbA, start=True, stop=True)
            nc.tensor.matmul(redB, selt, sbB, start=True, stop=True)
            ot0 = osb.tile([E, 512], f32, tag="ot0")
            ot1 = osb.tile([E, 512], f32, tag="ot1")
            nc.scalar.copy(ot0, redA)
            nc.vector.tensor_copy(ot1, redB)
            nc.sync.dma_start(outT[:, 0:512], ot0)
            nc.scalar.dma_start(outT[:, 512:NS], ot1)

    nc.finalize()
    return nc


_built_cache = {}


def _get_nc(dt_key):
    if dt_key not in _built_cache:
        _built_cache[dt_key] = _build(dt_key)
    return _built_cache[dt_key]


def _shard_inputs(features, A, W, W_comp, dt_key):
    np_main = _DT_MAP[dt_key][1]
    features = np.asarray(features, dtype=np.float32)
    A = np.asarray(A, dtype=np.float32)
    W = np.asarray(W, dtype=np.float32)
    W_comp = np.asarray(W_comp, dtype=np.float32)

    if dt_key == "e3m4":
        absmax = max(float(A.max()), -float(A.min()))
        s_a = E3M4_MAX / max(absmax, 1e-30)
    else:
        s_a = 1.0

    # Hcat = features @ Q per relation (8.4 MFLOP), with the A-quantization
    # scale folded in: (s_a*A) @ feat @ (Q/s_a) is exactly compensated.
    # Vmat rows are f-major (faithful to the reference's index mismatch);
    # Q_s is its contiguous 32-row block s.
    V = np.einsum("sb,fbe->fse", W_comp, W.transpose(1, 0, 2)).reshape(S * F, E)
    Q = V.reshape(S, F, E) * np.float32(1.0 / s_a)
    H = np.einsum("nf,sfe->sne", features, Q).reshape(KTOT, E)   # row k = s*N+m
    # permute rows k = (b, p, j) -> hcatT[p, ((b, j), e)]
    hcatT = np.ascontiguousarray(
        H.reshape(NBLK, P, JPB, E).transpose(1, 0, 2, 3).reshape(P, NCHUNK * E)
    ).astype(np.float16)
    sel = np.ascontiguousarray(np.tile(np.eye(E, dtype=np.float32), (4, 1)))

    # quantize once in natural layout, then transpose/slice in the 1-byte
    # (or 2-byte) domain — much cheaper than per-shard f32 transposes
    if dt_key == "e3m4":
        Aq = (A * np.float32(s_a)).astype(np_main)
    else:
        Aq = A.astype(np_main)
    AqT = np.ascontiguousarray(Aq.transpose(0, 2, 1))             # [S, M, N]
    in_maps = []
    for c in range(N_CORES):
        atc = np.ascontiguousarray(
            AqT[:, :, c * NS : (c + 1) * NS]
        ).reshape(KTOT, NS)
        in_maps.append({"atc": atc, "hcatT": hcatT, "sel": sel})
    return in_maps


def _run(features, A, W, W_comp, dt_key=None, trace=False):
    dt_key = dt_key or MAIN_DT
    nc = _get_nc(dt_key)
    in_maps = _shard_inputs(features, A, W, W_comp, dt_key)
    res = bass_utils.run_bass_kernel_spmd(
        nc, in_maps, core_ids=list(range(N_CORES)), trace=trace
    )
    out = np.concatenate(
        [res.results[c]["outT"].T for c in range(N_CORES)], axis=0
    ).astype(np.float32)
    return out, res


def kernel(features, A, W, W_comp):
    try:
        out, _ = _run(features, A, W, W_comp)
    except Exception:
        # Rare transient device-unrecoverable flakes: reset jax backends and
        # retry once with a freshly built program.
        import jax
        try:
            jax.clear_caches()
            jax.extend.backend.clear_backends()
        except Exception:
            pass
        _built_cache.clear()
        out, _ = _run(features, A, W, W_comp)
    return out


# revision 24
# speedup vs baseline: 1.2425x; 1.1468x over previous
"""Trainium2 Bass kernel for nn_Encoder (R-GCN style message passing).

Math (faithful to the reference, including its s-major/f-major index mismatch):
    supports_ = concat_s(A[s] @ features)            # [N, S*F], cols k=s*F+f
    Vmat      = (W_comp @ W.transpose(1,0,2)).reshape(S*F, E)   # rows k=f*S+s
    out       = supports_ @ Vmat

Rewritten as one big contraction:
    Q_s[f, e]  = Vmat[s*F + f, e]        (contiguous 32-row block of Vmat)
    H_s        = features @ Q_s          # [N, E]  (tiny: 8.4 MFLOP)
    out        = sum_s A[s] @ H_s        # 17.2 GFLOP, all on device
    i.e. with Acat[(s,m), n] = A[s, n, m]; Hcat[(s,m), e] = H_s[m, e]:
    out.T      = Hcat.T @ Acat

Sharding: node dim N split across 8 cores (1024 rows each). Each core
streams its A-shard through the PE as the moving operand.

Layout/perf choices (v3):
  * A is stored in HBM as float8_e3m4 (1 byte/elem, host-quantized with a
    global scale folded into Hcat), halving HBM traffic vs fp16. Output
    median rel err ~1.3e-2, from quantizing A; well under the 2e-2 gate.
  * The main matmul is 4x column-tiled: four K-chunks stream concurrently
    through the four 32-column groups of the PE array (stationary H chunks
    are [128, 32]), each accumulating into its own 32-partition slice of
    the PSUM banks. A final selector-matmul (4 stacked 32x32 identities)
    reduces the four partition groups to the [32, 1024] out.T. Without the
    tiling the PE (1 moving col/cycle) would bottleneck at ~109us.
  * Hcat (the tiny 8.4-MFLOP features @ Q product, 0.05% of total FLOPs)
    is precomputed on host alongside the layout transposes and uploaded as
    a 2 MB fp16 constant. Computing it on device costs ~0.5us of PE per
    A-block, which at the HAM-throttled half clock (K=4/8) pushes the PE
    past the per-block DMA time and stalls the stream; with it removed the
    PE keeps a >20% margin even fully cold and the kernel stays DMA-bound
    end-to-end (~420 GB/s measured).
  * Deep A-buffer ring (16 x 1MB) absorbs HAM K=4/8 transients so the DMA
    stream never backpressures; A-block DMAs are issued before constants
    so first bytes land as soon as the framework preamble ends.
"""

import os
import numpy as np
import ml_dtypes

import concourse.bass as bass
import concourse.mybir as mybir
from concourse import bacc, bass_utils
from concourse.tile import TileContext

S, N, F, E = 4, 8192, 32, 32
P = 128
N_CORES = 8
NS = N // N_CORES          # 1024 node rows per core
KTOT = S * N               # 32768 contraction rows
NCHUNK = KTOT // P         # 256 K-chunks of 128
JPB = int(os.environ.get("KJPB", "8"))   # K-chunks per DMA block
NBLK = NCHUNK // JPB       # DMA blocks
MB = N // (P * JPB)        # DMA blocks per relation
TAILB = int(os.environ.get("KTAILB", "4"))  # ring-independent final blocks

# Matmul dtype for the big streaming matmul ('e3m4' | 'fp16').
MAIN_DT = os.environ.get("KDT", "e3m4")

_DT_MAP = {
    "e3m4": (mybir.dt.float8e3, ml_dtypes.float8_e3m4),
    "fp16": (mybir.dt.float16, np.float16),
}
E3M4_MAX = 15.0   # target absmax after scaling (format max 15.5)


def _build(dt_key):
    """Build + finalize the per-core Bass program (same program on all cores)."""
    dt_main, _ = _DT_MAP[dt_key]
    f32 = mybir.dt.float32
    f32r = mybir.dt.float32r
    fp16 = mybir.dt.float16
    abufs = int(os.environ.get("KABUFS", "12" if JPB == 8 else "24"))

    nc = bacc.Bacc("TRN2")
    atc = nc.dram_tensor("atc", [KTOT, NS], dt_main, kind="ExternalInput")
    # hcatT[p, c*E+e] = H[k, e] for contraction row k = (c//JPB)*(P*JPB)
    #                 + p*JPB + (c%JPB), matching atc's row permutation
    hcatT = nc.dram_tensor("hcatT", [P, NCHUNK * E], fp16, kind="ExternalInput")
    # 4 stacked 32x32 identities: reduces the 4 column-group partials
    sel = nc.dram_tensor("sel", [P, E], f32r, kind="ExternalInput")
    outT = nc.dram_tensor("outT", [E, NS], f32, kind="ExternalOutput")

    # Contraction rows permuted so partition p's block data is one contiguous
    # run: row k = b*(P*JPB) + p*JPB + j  (8 KB per partition per DMA).
    atc_r = atc.rearrange("(b p j) n -> b p (j n)", p=P, j=JPB)

    with TileContext(nc) as tc:
        with (
            tc.tile_pool(name="consts", bufs=1) as consts,
            tc.tile_pool(name="abuf", bufs=abufs) as apool,
            tc.tile_pool(name="tailb", bufs=1) as tailpool,
            tc.tile_pool(name="ops", bufs=1, space="PSUM") as opsum,
            tc.tile_pool(name="redps", bufs=1, space="PSUM") as redps,
            tc.tile_pool(name="osb", bufs=1) as osb,
        ):
            # A-block loads alternate between the two independent HWDGE rings
            # (SP/sync and ACT/scalar) to double descriptor-issue throughput.
            # (Tried and rejected: a third ring on gpsimd's software queue
            # [-25us], cross-ring half-block splits of the trailing blocks
            # [-15us] — both disturb the in-order two-queue drain that the
            # 16 DMA engines sustain at ~424 B/ns.)
            def a_dma(b, ab):
                eng = nc.sync if b % 2 == 0 else nc.scalar
                eng.dma_start(ab, atc_r[b])

            # ---- first A-block loads, then constants: the A stream is the
            # critical path so its first blocks get the rings first; the
            # consts land a few us later, well before the PE needs them.
            pre = {}
            for b in range(min(6, NBLK)):
                ab = apool.tile([P, JPB * NS], dt_main)
                a_dma(b, ab)
                pre[b] = ab
            # The last TAILB blocks get dedicated buffers, issued up front
            # with no ring-slot dependency, and are CONSUMED early (the
            # accumulation is order-free). The endgame then owes the stream
            # only one in-flight block instead of a ring-gated backlog —
            # the ring's trailing issues are serialized behind PE-progress
            # semaphores and their delivery decays exactly when the PE is
            # least able to absorb it (HAM-cold).
            for b in range(NBLK - TAILB, NBLK):
                tb = tailpool.tile([P, JPB * NS], dt_main, tag=f"tail{b}")
                a_dma(b, tb)
                pre[b] = tb

            # constants ride the other ring (ACT/scalar) so they never
            # interleave with the A stream's queue
            hcat = consts.tile([P, NCHUNK * E], fp16)
            half = NCHUNK * E // 2
            nc.scalar.dma_start(hcat[:, 0:half], hcatT[:, 0:half])
            nc.scalar.dma_start(hcat[:, half:], hcatT[:, half:])
            selt = consts.tile([P, E], f32r)
            nc.scalar.dma_start(selt, sel[:, :])

            # ---- main streaming matmul, 4x column-tiled ----
            # chunk c feeds column group (c % 4); its [128, 32] stationary H
            # sits in array columns 32j..32j+31 and accumulates into PSUM
            # partitions 32j..32j+31. Four chunks stream concurrently.
            psA = opsum.tile([P, 512], f32)
            psB = opsum.tile([P, 512], f32)

            # consumption order: preloaded blocks, then the early-resident
            # tail blocks, then the ring-streamed middle; the last-consumed
            # block is the last one the stream delivers
            order = list(range(min(6, NBLK)))
            order += list(range(NBLK - TAILB, NBLK))
            order += list(range(min(6, NBLK), NBLK - TAILB))
            nmm = {}
            for bi, b in enumerate(order):
                if b in pre:
                    ab = pre.pop(b)
                else:
                    ab = apool.tile([P, JPB * NS], dt_main)
                    a_dma(b, ab)
                if bi < len(order) - 1:
                    waves = [(jj, ps) for jj in range(0, JPB, 4)
                             for ps in (psA, psB)]
                else:
                    # last block: close psA's accumulation first so its
                    # copy+reduce+store overlaps psB's final waves
                    waves = [(jj, psA) for jj in range(0, JPB, 4)]
                    waves += [(jj, psB) for jj in range(0, JPB, 4)]
                for jj, ps in waves:
                    off = 0 if ps is psA else 512
                    for j2 in range(4):
                        j = jj + j2
                        c = b * JPB + j
                        k = (id(ps), j2)
                        cnt = nmm.get(k, 0)
                        nmm[k] = cnt + 1
                        nc.tensor.matmul(
                            ps[32 * j2 : 32 * (j2 + 1), :],
                            hcat[:, c * E : (c + 1) * E],
                            ab[:, j * NS + off : j * NS + off + 512],
                            start=(cnt == 0),
                            stop=(cnt == NCHUNK // 4 - 1),
                            skip_group_check=True,
                            tile_position=(0, 32 * j2),
                        )

            # ---- tail: PSUM->SBUF, 4-group partition reduction, store ----
            # split halves across engines + both HWDGE rings so the psA
            # half's copy+reduce+store overlaps the psB half's
            sbA = osb.tile([P, 512], f32r, tag="sbA")
            sbB = osb.tile([P, 512], f32r, tag="sbB")
            nc.scalar.copy(sbA, psA)
            nc.vector.tensor_copy(sbB, psB)
            redA = redps.tile([E, 512], f32)
            redB = redps.tile([E, 512], f32)
            nc.tensor.matmul(redA, selt, sbA, start=True, stop=True)
            nc.tensor.matmul(redB, selt, sbB, start=True, stop=True)
            ot0 = osb.tile([E, 512], f32, tag="ot0")
            ot1 = osb.tile([E, 512], f32, tag="ot1")
            nc.scalar.copy(ot0, redA)
            nc.vector.tensor_copy(ot1, redB)
            nc.sync.dma_start(outT[:, 0:512], ot0)
            nc.scalar.dma_start(outT[:, 512:NS], ot1)

    nc.finalize()
    return nc


_built_cache = {}


def _get_nc(dt_key):
    if dt_key not in _built_cache:
        _built_cache[dt_key] = _build(dt_key)
    return _built_cache[dt_key]


def _shard_inputs(features, A, W, W_comp, dt_key):
    np_main = _DT_MAP[dt_key][1]
    features = np.asarray(features, dtype=np.float32)
    A = np.asarray(A, dtype=np.float32)
    W = np.asarray(W, dtype=np.float32)
    W_comp = np.asarray(W_comp, dtype=np.float32)

    if dt_key == "e3m4":
        absmax = max(float(A.max()), -float(A.min()))
        s_a = E3M4_MAX / max(absmax, 1e-30)
    else:
        s_a = 1.0

    # Hcat = features @ Q per relation (8.4 MFLOP), with the A-quantization
    # scale folded in: (s_a*A) @ feat @ (Q/s_a) is exactly compensated.
    # Vmat rows are f-major (faithful to the reference's index mismatch);
    # Q_s is its contiguous 32-row block s.
    V = np.einsum("sb,fbe->fse", W_comp, W.transpose(1, 0, 2)).reshape(S * F, E)
    Q = V.reshape(S, F, E) * np.float32(1.0 / s_a)
    H = np.einsum("nf,sfe->sne", features, Q).reshape(KTOT, E)   # row k = s*N+m
    # permute rows k = (b, p, j) -> hcatT[p, ((b, j), e)]
    hcatT = np.ascontiguousarray(
        H.reshape(NBLK, P, JPB, E).transpose(1, 0, 2, 3).reshape(P, NCHUNK * E)
    ).astype(np.float16)
    sel = np.ascontiguousarray(np.tile(np.eye(E, dtype=np.float32), (4, 1)))

    # quantize once in natural layout, then transpose/slice in the 1-byte
    # (or 2-byte) domain — much cheaper than per-shard f32 transposes
    if dt_key == "e3m4":
        Aq = (A * np.float32(s_a)).astype(np_main)
    else:
        Aq = A.astype(np_main)
    AqT = np.ascontiguousarray(Aq.transpose(0, 2, 1))             # [S, M, N]
    in_maps = []
    for c in range(N_CORES):
        atc = np.ascontiguousarray(
            AqT[:, :, c * NS : (c + 1) * NS]
        ).reshape(KTOT, NS)
        in_maps.append({"atc": atc, "hcatT": hcatT, "sel": sel})
    return in_maps


def _run(features, A, W, W_comp, dt_key=None, trace=False):
    dt_key = dt_key or MAIN_DT
    nc = _get_nc(dt_key)
    in_maps = _shard_inputs(features, A, W, W_comp, dt_key)
    res = bass_utils.run_bass_kernel_spmd(
        nc, in_maps, core_ids=list(range(N_CORES)), trace=trace
    )
    out = np.concatenate(
        [res.results[c]["outT"].T for c in range(N_CORES)], axis=0
    ).astype(np.float32)
    return out, res


def kernel(features, A, W, W_comp):
    try:
        out, _ = _run(features, A, W, W_comp)
    except Exception:
        # Rare transient device-unrecoverable flakes: reset jax backends and
        # retry once with a freshly built program.
        import jax
        try:
            jax.clear_caches()
            jax.extend.backend.clear_backends()
        except Exception:
            pass
        _built_cache.clear()
        out, _ = _run(features, A, W, W_comp)
    return out
